# revision 23
# baseline (speedup 1.0000x reference)
"""3-layer GAT (graph attention network) on Trainium2 — Bass/Tile, 8-core SPMD.

Sharding: nodes are partitioned into 8 contiguous ranges (graph/data
parallel).  Each core owns the edges whose *destination* falls in its range.

All per-core node data lives in "slot" order: destination nodes are packed
into ng groups of 128 slots (group g covers a window of <=128 consecutive
nodes); slot = g*128 + (node - group_base).  The host permutes the input
features into slot order, the epilogue writes outputs in slot order (so the
next layer's phase A needs NO gather), and per-group attention tables live
at static slot addresses (shared SPMD program across cores).

Per layer:
  phase A : one PE matmul per 128-slot block against [W | War] gives feat
            and er.  Feat rows (bf16, 256 B) form the gather table (written
            in two halves so the first AllGather fires at phase-A midpoint);
            er (8 B/slot) goes to a packed local table.
  2x AllGather (half tables) so every core can gather arbitrary src rows.
  edge    : per chunk, feat[src] rows are dma_gathered with indices split
            over all 4 SWDGE queues (desc-gen runs on a DSP pair per queue).
            One-hot matrices (host-built fp8, streamed from DRAM — both
            normal and transposed layouts) drive PE matmuls:
              er_edge = ohT_t.T @ er_blk          (per tile, [128,H])
              [msg | den] = oh_t.T @ [w*feat | w] (accumulated per group)
            el is computed on device (feat*al, reduce over D); w =
            exp(leaky_relu(el+er)) via one fused DVE op + scalar Exp.
            Epilogue divides by den (edge softmax), adds bias, relu.
Edge softmax skips the segment-max subtraction: alpha = exp(e)/sum(exp(e))
is mathematically identical and the logits here are O(1).

dma_gather uses int16 indices (max 32767): edges are split per group into
zone "lo" (src in the first half of its core's slots, gathered from the
half-A table) and zone "hi" (second half, half-B table); 8*SLOTS/2 = 26624
rows per half table, so all indices fit int16 with no offset views.
Edges are sorted by src within each (group, zone) for HBM locality.
"""

import numpy as np

try:
    import ml_dtypes
    _BF16 = ml_dtypes.bfloat16
    _FP8 = ml_dtypes.float8_e4m3
except ImportError:  # pragma: no cover
    _BF16 = None
    _FP8 = None

# ---------------- static problem config (self-contained) ---------------------
N_CORES = 8
NEG_SLOPE = 0.2
P = 128
GROUP_E = 2048             # max edges per PSUM accumulation group
CHUNK_GROUPS = 2           # groups per gather chunk
# (in_dim, H, D, apply_relu) per layer
LAYERS = [(128, 4, 32, True), (128, 4, 32, True), (128, 1, 64, False)]
OUT_DIM = 64
ROW = 128                  # table row, bf16 elems (256 B): [feat | pad]
HMAX = 4                   # er table row width (elems)

_cache = {}
last_run_info = {}


# ============================ host-side preprocessing ========================

def _wrap16(vals, cols):
    """dma_gather index layout: entry i -> [i % 16, i // 16], replicated
    across the 8 groups of 16 partitions."""
    t = np.zeros((16, cols), np.int16)
    n = len(vals)
    t[np.arange(n) % 16, np.arange(n) // 16] = vals.astype(np.int16)
    return np.tile(t, (8, 1))


def _preprocess(src, dst, n_nodes, n_cores):
    npc = n_nodes // n_cores
    cores = []
    for c in range(n_cores):
        lo = c * npc
        m = (dst >= lo) & (dst < lo + npc)
        s = src[m].astype(np.int64)
        d = (dst[m] - lo).astype(np.int64)
        o = np.argsort(d, kind="stable")
        s, d = s[o], d[o]
        counts = np.bincount(d, minlength=npc)
        cum = np.zeros(npc + 1, np.int64)
        np.cumsum(counts, out=cum[1:])
        groups = []
        base = 0
        while base < npc:
            dmax = min(base + P, npc)
            limit = cum[base] + GROUP_E
            dend = int(np.searchsorted(cum, limit, side="right")) - 1
            dend = min(dend, dmax)
            if dend <= base:
                raise ValueError(f"dst {base} has degree > {GROUP_E}")
            groups.append((base, int(cum[base]), int(cum[dend])))
            base = dend
        cores.append((s, d, groups))

    ng = max(len(g) for (_, _, g) in cores)
    lcm = (2 * CHUNK_GROUPS) if CHUNK_GROUPS % 2 else CHUNK_GROUPS
    ng = ((ng + lcm - 1) // lcm) * lcm   # even and a chunk multiple
    SLOTS = ng * P
    ng2 = ng // 2
    S2 = ng2 * P                          # half-table rows per core
    # node -> slot map per core
    node_slot = []
    for c, (s, d, groups) in enumerate(cores):
        srow = np.zeros(npc, np.int64)
        for gi, (b, e0, e1) in enumerate(groups):
            b_next = groups[gi + 1][0] if gi + 1 < len(groups) else npc
            srow[b:b_next] = gi * P + (np.arange(b, b_next) - b)
        node_slot.append(srow)
    # zone (0 = first half of owner core's slots) and half-table index
    zidx = np.empty(n_nodes, np.int64)
    zzone = np.empty(n_nodes, np.bool_)
    for c in range(n_cores):
        sl = node_slot[c]
        hi = sl >= S2
        zzone[c * npc:(c + 1) * npc] = hi
        zi = np.where(hi, c * S2 + (sl - S2), c * S2 + sl)
        zidx[c * npc:(c + 1) * npc] = zi
    assert n_cores * S2 <= 32768

    TL = np.zeros(ng, np.int64)
    TH = np.zeros(ng, np.int64)
    for (s, d, groups) in cores:
        hz = zzone[s]
        for gi, (b, e0, e1) in enumerate(groups):
            nhi = int(hz[e0:e1].sum())
            nlo = (e1 - e0) - nhi
            TL[gi] = max(TL[gi], (nlo + P - 1) // P)
            TH[gi] = max(TH[gi], (nhi + P - 1) // P)
    lo_base = np.zeros(ng + 1, np.int64)
    hi_base = np.zeros(ng + 1, np.int64)
    np.cumsum(TL, out=lo_base[1:])
    np.cumsum(TH, out=hi_base[1:])
    SL = int(lo_base[ng]) * P
    SH = int(hi_base[ng]) * P
    SLP = max(SL, 2048)
    SHP = max(SH, 2048)
    CT = SL // P + SH // P               # total tiles, chunk-major layout

    per_core = []
    for c, (s, d, groups) in enumerate(cores):
        gz = zzone[s]
        gi_idx = zidx[s]
        idx_lo = np.zeros(SL, np.int64)
        idx_hi = np.zeros(SH, np.int64)
        dr_cm = np.full(CT * P, -1.0, np.float32)
        for gi, (b, e0, e1) in enumerate(groups):
            eg_i = gi_idx[e0:e1]
            eg_d = d[e0:e1]
            hm = gz[e0:e1]
            g0 = (gi // CHUNK_GROUPS) * CHUNK_GROUPS
            g1 = min(g0 + CHUNK_GROUPS, ng)
            cm0 = int(lo_base[g0] + hi_base[g0])
            ltc = int(lo_base[g1] - lo_base[g0])
            for zone, msk in ((0, ~hm), (1, hm)):
                zs = eg_i[msk]
                zd = eg_d[msk]
                o2 = np.argsort(zs, kind="stable")    # src-sorted for HBM
                zs, zd = zs[o2], zd[o2]
                n = len(zs)
                if zone == 0:
                    o = int(lo_base[gi]) * P
                    e_end = int(lo_base[gi + 1]) * P
                    idx_lo[o:o + n] = zs
                    idx_lo[o + n:e_end] = 0       # (-1 skip disabled)
                    if n == 0 and e_end > o:
                        idx_lo[o] = 0
                    cmo = (cm0 + int(lo_base[gi] - lo_base[g0])) * P
                else:
                    o = int(hi_base[gi]) * P
                    e_end = int(hi_base[gi + 1]) * P
                    idx_hi[o:o + n] = zs
                    idx_hi[o + n:e_end] = 0
                    if n == 0 and e_end > o:
                        idx_hi[o] = 0
                    cmo = (cm0 + ltc + int(hi_base[gi] - hi_base[g0])) * P
                dr_cm[cmo:cmo + n] = (zd - b).astype(np.float32)

        drt = dr_cm.reshape(CT, P)                    # [tile, e]
        eye = (drt[:, :, None] ==
               np.arange(P, dtype=np.float32)[None, None, :])
        oh = eye.astype(_FP8)                         # [tile, e, j]
        oh_d = np.ascontiguousarray(
            oh.transpose(1, 0, 2).reshape(P, CT * P))  # [e, tile*j]
        ohT_d = np.ascontiguousarray(
            oh.transpose(2, 0, 1).reshape(P, CT * P))  # [j, tile*e]

        def _padcols(a, cols):
            out = np.zeros((a.shape[0], cols), a.dtype)
            out[:, :a.shape[1]] = a
            return out

        # pack [oh_chunk | ohT_chunk] contiguously per chunk
        ohb = np.zeros((P, 2 * CT * P), _FP8)
        for g0 in range(0, ng, CHUNK_GROUPS):
            g1 = g0 + CHUNK_GROUPS
            c0 = int(lo_base[g0] + hi_base[g0])
            c1 = int(lo_base[g1] + hi_base[g1])
            ohb[:, 2 * c0 * P:(c0 + c1) * P] = oh_d[:, c0 * P:c1 * P]
            ohb[:, (c0 + c1) * P:2 * c1 * P] = ohT_d[:, c0 * P:c1 * P]
        per_core.append(dict(
            idx_lo=_padcols(_wrap16(idx_lo, max(SL // 16, 1)), SLP // 16),
            idx_hi=_padcols(_wrap16(idx_hi, max(SH // 16, 1)), SHP // 16),
            ohb=ohb,
            srow=node_slot[c],
        ))
    mzt = 1
    for g0 in range(0, ng, CHUNK_GROUPS):
        g1 = g0 + CHUNK_GROUPS
        mzt = max(mzt, int(lo_base[g1] - lo_base[g0]),
                  int(hi_base[g1] - hi_base[g0]))
    meta = dict(ng=ng, TL=tuple(int(x) for x in TL),
                TH=tuple(int(x) for x in TH), SL=SL, SH=SH,
                SLP=SLP, SHP=SHP, CT=CT, MZT=mzt,
                npc=npc, n_nodes=n_nodes, n_cores=n_cores)
    return meta, per_core


# ============================ device program =================================

def _build_program(meta):
    import concourse.bass as bass
    import concourse.tile as tile
    from concourse import bacc, mybir

    f32 = mybir.dt.float32
    bf16 = mybir.dt.bfloat16
    fp8 = mybir.dt.float8e4
    i16 = mybir.dt.int16
    AF = mybir.ActivationFunctionType
    OP = mybir.AluOpType

    ng, SL, SH = meta["ng"], meta["SL"], meta["SH"]
    SLP, SHP, CT = meta["SLP"], meta["SHP"], meta["CT"]
    MAXZT = meta["MZT"]
    TL, TH = meta["TL"], meta["TH"]
    npc = meta["npc"]
    n_cores = meta["n_cores"]
    SLOTS = ng * P
    ng2 = ng // 2
    S2 = ng2 * P
    lo_base = np.concatenate([[0], np.cumsum(TL)]).astype(int)
    hi_base = np.concatenate([[0], np.cumsum(TH)]).astype(int)
    nchunk = ng // CHUNK_GROUPS

    nc = bacc.Bacc("TRN2", target_bir_lowering=False, debug=False,
                   enable_asserts=False, num_devices=n_cores,
                   num_swdge_queues=4)

    qctr = [0]

    def _gather2(out_ap3, in_ap, idxs2, t0, t1, elem, cuts):
        """gather tiles [t0,t1) of a zone, one call per group (cuts =
        group tile boundaries) on rotating queues; trailing -1 indices in
        each call's range are skipped by the DGE (padding elided)."""
        bounds = [t0] + [c for c in cuts if t0 < c < t1] + [t1]
        for a, b in zip(bounds[:-1], bounds[1:]):
            nc.gpsimd.dma_gather(
                out_ap=out_ap3[:, a - t0:b - t0, :],
                in_ap=in_ap,
                idxs_ap=idxs2[:, a * 8:b * 8],
                num_idxs=(b - a) * P,
                num_idxs_reg=(b - a) * P,
                elem_size=elem,
                single_packet=False,
                queue_num=qctr[0] % 4,
            )
            qctr[0] += 1

    t_feats = nc.dram_tensor("features_own", [SLOTS, 128], bf16,
                             kind="ExternalInput").ap()
    t_idx_lo = nc.dram_tensor("idx_lo", [P, SLP // 16], i16,
                              kind="ExternalInput").ap()
    t_idx_hi = nc.dram_tensor("idx_hi", [P, SHP // 16], i16,
                              kind="ExternalInput").ap()
    t_ohb = nc.dram_tensor("ohb", [P, 2 * CT * P], fp8,
                           kind="ExternalInput").ap()
    t_ident = nc.dram_tensor("identity", [P, P], f32,
                             kind="ExternalInput").ap()
    t_WW, t_b, t_al = [], [], []
    for li, (ind, H, D, _) in enumerate(LAYERS):
        hd = H * D
        t_WW.append(nc.dram_tensor(f"WW{li}", [ind, hd + H], bf16,
                                   kind="ExternalInput").ap())
        t_b.append(nc.dram_tensor(f"br{li}", [P, hd], f32,
                                  kind="ExternalInput").ap())
        t_al.append(nc.dram_tensor(f"albc{li}", [P, MAXZT * hd], bf16,
                                   kind="ExternalInput").ap())
    t_out = nc.dram_tensor("out", [SLOTS, OUT_DIM], f32,
                           kind="ExternalOutput").ap()

    with tile.TileContext(nc) as tc:
        with (
            tc.tile_pool(name="const", bufs=1) as cpool,
            tc.tile_pool(name="big", bufs=1) as bigpool,
            tc.tile_pool(name="sb", bufs=3) as sb,
            tc.tile_pool(name="fg", bufs=4) as fgpool,
            tc.tile_pool(name="wp", bufs=2) as wpool,
            tc.tile_pool(name="ps", bufs=2, space="PSUM") as pspool,
            tc.tile_pool(name="per", bufs=2, space="PSUM") as perpool,
            tc.tile_pool(name="psA", bufs=2, space="PSUM") as psA,
            tc.tile_pool(name="psB", bufs=2, space="PSUM") as psB,
            tc.tile_pool(name="dram", bufs=1, space="DRAM") as dram,
        ):
            # ---- constants ----
            ident = cpool.tile([P, P], f32)
            nc.sync.dma_start(ident[:], t_ident)
            ident_bf = cpool.tile([P, P], bf16, tag="ident_bf")
            nc.vector.tensor_copy(ident_bf[:], ident[:])
            idx_lo = cpool.tile([P, SLP // 16], i16)
            nc.sync.dma_start(idx_lo[:], t_idx_lo)
            idx_hi = cpool.tile([P, SHP // 16], i16)
            nc.sync.dma_start(idx_hi[:], t_idx_hi)
            zer = cpool.tile([P, 2 * MAXZT * HMAX], bf16, tag="zer")
            nc.vector.memset(zer[:], 0.0)
            WWs, Bs, ALs = [], [], []
            for li, (ind, H, D, _) in enumerate(LAYERS):
                hd = H * D
                w = cpool.tile([ind, hd + H], bf16, tag=f"WW{li}")
                nc.sync.dma_start(w[:], t_WW[li])
                WWs.append(w)
                bb = cpool.tile([P, hd], f32, tag=f"br{li}")
                nc.sync.dma_start(bb[:], t_b[li])
                Bs.append(bb)
                aa = cpool.tile([P, MAXZT * 128], bf16, tag="albc",
                                bufs=2, name=f"albc{li}")
                nc.sync.dma_start(aa[:, :MAXZT * hd], t_al[li])
                ALs.append(aa)

            # per-layer phase-A state: (tabsb pair, er pair, halves)
            state = {}

            def alloc_phase_a(li):
                tabsb = [bigpool.tile([P, ng2 * ROW], bf16,
                                      tag=f"tabsb{hf}", name=f"tabsb{hf}")
                         for hf in range(2)]
                ers = [cpool.tile([P, ng2 * HMAX], bf16, tag=f"er_own{hf}",
                                  bufs=2, name=f"er_own{hf}")
                       for hf in range(2)]
                state[li] = dict(tabsb=tabsb, ers=ers, halves=[None, None])

            def phase_a_group(li, g, x_src):
                """compute feat/er for 128-slot block g of layer li"""
                ind, H, D, _ = LAYERS[li]
                hd = H * D
                st = state[li]
                hf, i2 = divmod(g, ng2)
                xT_ps = psA.tile([P, P], bf16, tag="psA", name="xT_ps")
                nc.tensor.transpose(
                    out=xT_ps[:], in_=x_src[:, g * ind:(g + 1) * ind],
                    identity=ident_bf[:])
                xT = sb.tile([P, ind], bf16, tag="xT", name="xT")
                nc.vector.tensor_copy(xT[:], xT_ps[:, :ind])
                f_ps = psB.tile([P, hd + H], f32, tag="psB", name="f_ps")
                nc.tensor.matmul(out=f_ps[:], lhsT=xT[:],
                                 rhs=WWs[li][:], start=True, stop=True)
                nc.vector.tensor_copy(
                    st["tabsb"][hf][:, i2 * ROW:i2 * ROW + hd], f_ps[:, :hd])
                nc.any.tensor_copy(
                    st["ers"][hf][:, i2 * HMAX:i2 * HMAX + H],
                    f_ps[:, hd:hd + H])

            def emit_ag(li, hf):
                """store half table to DRAM + AllGather it"""
                st = state[li]
                tab_own = dram.tile([S2, ROW], bf16, tag=f"tab_own{li}_{hf}",
                                    name=f"tab_own{li}_{hf}")
                nc.sync.dma_start(
                    tab_own[:].rearrange("(i p) d -> p i d", p=P),
                    st["tabsb"][hf][:].rearrange("p (d2 d) -> p d2 d", d=ROW))
                tab_full = dram.tile([n_cores * S2, ROW], bf16,
                                     addr_space="Shared",
                                     tag=f"tab_full{li}_{hf}",
                                     name=f"tab_full{li}_{hf}")
                if n_cores == 1:
                    nc.sync.dma_start(tab_full[:], tab_own[:])
                else:
                    nc.gpsimd.collective_compute(
                        "AllGather", mybir.AluOpType.bypass,
                        replica_groups=[list(range(n_cores))],
                        ins=[tab_own[:]],
                        outs=[tab_full[:]],
                    )
                st["halves"][hf] = tab_full

            # zero the fg rotation buffers once: rows elided by -1
            # indices must never expose uninitialized SBUF (inf/nan)
            ctmax = max(int(lo_base[g + CHUNK_GROUPS] - lo_base[g])
                        + int(hi_base[g + CHUNK_GROUPS] - hi_base[g])
                        for g in range(0, ng, CHUNK_GROUPS))
            for _ in range(4):
                fgz = fgpool.tile([P, ctmax * ROW], bf16, tag="fg",
                                  name="fgz")
                nc.vector.memset(fgz[:], 0.0)

            # ---- layer 0 phase A (upfront) ----
            x_cur = bigpool.tile([P, ng * 128], bf16, tag="x_own",
                                 bufs=2, name="x_own0")
            nc.sync.dma_start(
                x_cur[:].rearrange("p (i d) -> p i d", d=128),
                t_feats.rearrange("(i p) d -> p i d", p=P))
            alloc_phase_a(0)
            for g in range(ng):
                phase_a_group(0, g, x_cur)
                if g == ng2 - 1:
                    emit_ag(0, 0)
            emit_ag(0, 1)

            # ---- layers: edge phase with interleaved next-layer phase A ----
            for li, (ind, H, D, apply_relu) in enumerate(LAYERS):
                hd = H * D
                st = state[li]
                halves = st["halves"]
                ers = st["ers"]
                last = li == len(LAYERS) - 1
                if not last:
                    x_next = bigpool.tile([P, ng * 128], bf16,
                                          tag="x_own", bufs=2,
                                          name=f"x_own{li + 1}")
                    alloc_phase_a(li + 1)

                # ---- software-pipelined chunk emission ----
                # stage schedule at iteration k:
                #   loads(k) + er-matmuls(k)  [PE: er(k) BEFORE seg(k-1) so a
                #   stalled seg never blocks the next chunk's er spread]
                #   dve+seg+epilogue(k-1)
                #   phase-A(k-2) for the next layer (+AllGather marks)
                cs = {}

                def chunk_geom(k):
                    g0 = k * CHUNK_GROUPS
                    g1 = g0 + CHUNK_GROUPS
                    lt0, lt1 = int(lo_base[g0]), int(lo_base[g1])
                    ht0, ht1 = int(hi_base[g0]), int(hi_base[g1])
                    return g0, g1, lt0, lt1, ht0, ht1

                def ot_view(g):
                    if last:
                        return cs[g // CHUNK_GROUPS]["otc"][
                            :, (g % CHUNK_GROUPS) * hd:
                            (g % CHUNK_GROUPS + 1) * hd]
                    return x_next[:, g * hd:(g + 1) * hd]

                def emit_loads_er(k):
                    g0, g1, lt0, lt1, ht0, ht1 = chunk_geom(k)
                    ltc, htc = lt1 - lt0, ht1 - ht0
                    ct = ltc + htc
                    st_k = dict(ct=ct, ltc=ltc, htc=htc)
                    cs[k] = st_k
                    if last:
                        st_k["otc"] = sb.tile([P, CHUNK_GROUPS * hd], f32,
                                              tag="otc", name="otc")
                    if ct == 0:
                        return
                    cm0 = lt0 + ht0
                    ohp = fgpool.tile([P, 2 * ct * P], fp8, tag="ohp",
                                      name="ohp")
                    nc.sync.dma_start(
                        ohp[:], t_ohb[:, 2 * cm0 * P:2 * (cm0 + ct) * P])
                    st_k["ohp"] = ohp
                    fgt = fgpool.tile([P, ct * ROW], bf16, tag="fg",
                                      name="fg")
                    fg3 = fgt[:].rearrange("p (j d) -> p j d", d=ROW)
                    st_k["fg3"] = fg3
                    st_k["fgf"] = fgt
                    if ltc:
                        _gather2(fg3[:, :ltc, :], halves[0][:],
                                 idx_lo, lt0, lt1, ROW,
                                 [int(lo_base[g]) for g in range(g0 + 1, g1)])
                    if htc:
                        _gather2(fg3[:, ltc:, :], halves[1][:],
                                 idx_hi, ht0, ht1, ROW,
                                 [int(hi_base[g]) for g in range(g0 + 1, g1)])
                    # er spread via PE (one chunk ahead of seg matmuls)
                    tiles_of = {g: (list(range(int(lo_base[g]) - lt0,
                                               int(lo_base[g + 1]) - lt0))
                                    + list(range(
                                        ltc + int(hi_base[g]) - ht0,
                                        ltc + int(hi_base[g + 1]) - ht0)))
                                for g in range(g0, g1)}
                    st_k["tiles_of"] = tiles_of
                    ohT = ohp[:, ct * P:]
                    er_ps = perpool.tile([P, ct * H], f32, tag="er",
                                         name="er_ps")
                    for g in range(g0, g1):
                        ghf, gi2 = divmod(g, ng2)
                        erb = ers[ghf][:, gi2 * HMAX:gi2 * HMAX + H]
                        for ci in tiles_of[g]:
                            nc.tensor.matmul(
                                out=er_ps[:, ci * H:(ci + 1) * H],
                                lhsT=ohT[:, ci * P:(ci + 1) * P],
                                rhs=erb,
                                start=True, stop=True)
                    st_k["er_ps"] = er_ps

                def emit_dve(k):
                    st_k = cs[k]
                    ct, ltc, htc = st_k["ct"], st_k["ltc"], st_k["htc"]
                    if ct == 0:
                        g0, g1 = k * CHUNK_GROUPS, (k + 1) * CHUNK_GROUPS
                        for g in range(g0, g1):
                            ot = ot_view(g)
                            if apply_relu:
                                nc.vector.tensor_scalar_max(
                                    ot, Bs[li][:, :hd], 0.0)
                            else:
                                nc.vector.tensor_copy(ot, Bs[li][:, :hd])
                        return
                    fg3, er_ps = st_k["fg3"], st_k["er_ps"]
                    fgf = st_k["fgf"]
                    esum = wpool.tile([P, ct * H], f32, tag="esum",
                                      name="esum")
                    prod = wpool.tile([P, ct * hd], bf16, tag="prod",
                                      name="prod")
                    ph = wpool.tile([P, ct * hd // 2], bf16, tag="ph",
                                    name="ph")
                    D2 = D // 2
                    for (zoff, znt) in ((0, ltc), (ltc, htc)):
                        if znt == 0:
                            continue
                        if hd == ROW:
                            # fully contiguous 2D op (2x DVE mode eligible)
                            nc.vector.tensor_tensor(
                                out=prod[:, zoff * hd:(zoff + znt) * hd],
                                in0=fgf[:, zoff * ROW:(zoff + znt) * ROW],
                                in1=ALs[li][:, :znt * hd],
                                op=OP.mult)
                        else:
                            nc.vector.tensor_tensor(
                                out=prod[:, zoff * hd:(zoff + znt) * hd]
                                    .rearrange("p (j d) -> p j d", d=hd),
                                in0=fg3[:, zoff:zoff + znt, :hd],
                                in1=ALs[li][:, :znt * hd]
                                    .rearrange("p (j d) -> p j d", d=hd),
                                op=OP.mult)
                        # fold D -> D/2 with a 2x-capable add, then 1x reduce
                        pv = prod[:, zoff * hd:(zoff + znt) * hd] \
                            .rearrange("p (j d2) -> p j d2", d2=D)
                        nc.vector.tensor_tensor(
                            out=ph[:, zoff * hd // 2:(zoff + znt) * hd // 2]
                                .rearrange("p (j d) -> p j d", d=D2),
                            in0=pv[:, :, :D2],
                            in1=pv[:, :, D2:],
                            op=OP.add)
                        nc.vector.tensor_reduce(
                            out=esum[:, zoff * H:(zoff + znt) * H],
                            in_=ph[:, zoff * hd // 2:(zoff + znt) * hd // 2]
                                .rearrange("p (j h d) -> p (j h) d",
                                           h=H, d=D2),
                            axis=mybir.AxisListType.X, op=OP.add)
                    nc.vector.tensor_tensor(
                        out=esum[:], in0=esum[:], in1=er_ps[:], op=OP.add)
                    lrl = wpool.tile([P, ct * H], f32, tag="lrl",
                                     bufs=2, name="lrl")
                    nc.vector.scalar_tensor_tensor(
                        out=lrl[:], in0=esum[:], scalar=NEG_SLOPE,
                        in1=esum[:], op0=OP.mult, op1=OP.max)
                    wch = wpool.tile([P, ct * H], bf16, tag="w", bufs=2,
                                     name="w")
                    nc.scalar.activation(wch[:], lrl[:], AF.Exp)
                    mge = sb.tile([P, ct * (hd + H)], bf16, tag="mge",
                                  bufs=3, name="mge")
                    nc.vector.tensor_tensor(
                        out=mge[:].rearrange("p (t e) -> p t e",
                                             e=hd + H)[:, :, :hd]
                            .rearrange("p t (h d) -> p t h d", d=D),
                        in0=fg3[:, :, :hd]
                            .rearrange("p t (h d) -> p t h d", d=D),
                        in1=wch[:]
                            .rearrange("p (t h) -> p t h", h=H)
                            .to_broadcast([P, ct, H, D]),
                        op=OP.mult)
                    nc.vector.tensor_tensor(
                        out=mge[:].rearrange("p (t e) -> p t e",
                                             e=hd + H)[:, :, hd:],
                        in0=wch[:].rearrange("p (t h) -> p t h", h=H),
                        in1=zer[:, :ct * H]
                            .rearrange("p (t h) -> p t h", h=H),
                        op=OP.add)
                    st_k["mge"] = mge

                def emit_seg(k):
                    st_k = cs[k]
                    if st_k["ct"] == 0:
                        return
                    g0, g1 = k * CHUNK_GROUPS, (k + 1) * CHUNK_GROUPS
                    tiles_of, mge = st_k["tiles_of"], st_k["mge"]
                    oh = st_k["ohp"][:, :st_k["ct"] * P]
                    for g in range(g0, g1):
                        tg = tiles_of[g]
                        ot = ot_view(g)
                        if not tg:
                            if apply_relu:
                                nc.vector.tensor_scalar_max(
                                    ot, Bs[li][:, :hd], 0.0)
                            else:
                                nc.vector.tensor_copy(ot, Bs[li][:, :hd])
                            continue
                        ps = pspool.tile([P, hd + H], f32, tag="ps",
                                         name="ps")
                        for n, ci in enumerate(tg):
                            nc.tensor.matmul(
                                out=ps[:],
                                lhsT=oh[:, ci * P:(ci + 1) * P],
                                rhs=mge[:, ci * (hd + H):
                                        (ci + 1) * (hd + H)],
                                start=(n == 0),
                                stop=(n == len(tg) - 1))
                        den = sb.tile([P, H], f32, tag="den", name="den")
                        nc.vector.tensor_scalar_max(
                            den[:], ps[:, hd:hd + H], 1e-12)
                        rec = sb.tile([P, H], f32, tag="rec", name="rec")
                        nc.vector.reciprocal(rec[:], den[:])
                        nc.vector.tensor_tensor(
                            out=ot.rearrange("p (h d) -> p h d", d=D),
                            in0=ps[:, :hd].rearrange("p (h d) -> p h d",
                                                     d=D),
                            in1=rec[:].to_broadcast([P, H, D]),
                            op=OP.mult)
                        nc.vector.tensor_tensor(out=ot, in0=ot,
                                                in1=Bs[li][:, :hd],
                                                op=OP.add)
                        if apply_relu:
                            nc.vector.scalar_tensor_tensor(
                                out=ot, in0=ot, scalar=0.0, in1=ot,
                                op0=OP.mult, op1=OP.max)
                    if last:
                        nc.sync.dma_start(
                            t_out[g0 * P:g1 * P, :]
                            .rearrange("(i p) d -> p i d", p=P),
                            st_k["otc"][:].rearrange("p (i d) -> p i d",
                                                     d=hd))

                def emit_pa(k):
                    if last:
                        return
                    g0, g1 = k * CHUNK_GROUPS, (k + 1) * CHUNK_GROUPS
                    for g in range(g0, g1):
                        phase_a_group(li + 1, g, x_next)
                        if g == ng2 - 1:
                            emit_ag(li + 1, 0)
                        elif g == ng - 1:
                            emit_ag(li + 1, 1)

                for k in range(nchunk + 2):
                    if k < nchunk:
                        emit_loads_er(k)
                    if 1 <= k <= nchunk:
                        emit_dve(k - 1)
                        emit_seg(k - 1)
                    if k >= 2:
                        emit_pa(k - 2)
                if not last:
                    x_cur = x_next
    nc.compile()
    return nc


# ============================ entry point ====================================

def _meta_key(meta):
    return (meta["ng"], meta["TL"], meta["TH"], meta["SL"], meta["SH"],
            meta["npc"], meta["n_nodes"], meta["n_cores"])


def _get_compiled(meta):
    key = _meta_key(meta)
    if key not in _cache:
        _cache[key] = _build_program(meta)
    return _cache[key]


def _make_in_maps(inputs, meta, per_core):
    f32 = np.float32
    npc = meta["npc"]
    ng = meta["ng"]
    n_cores = meta["n_cores"]
    ident = np.eye(P, dtype=f32)
    common = {"identity": ident}
    for li in range(len(LAYERS)):
        ind, H, D, _ = LAYERS[li]
        W = np.asarray(inputs[f"W{li}"], f32)
        al = np.asarray(inputs[f"al{li}"], f32)
        ar = np.asarray(inputs[f"ar{li}"], f32)
        b = np.asarray(inputs[f"b{li}"], f32)
        hd = H * D
        ar_flat = np.zeros((hd, H), f32)
        for h in range(H):
            ar_flat[h * D:(h + 1) * D, h] = ar[h]
        WW = np.concatenate([W, (W @ ar_flat).astype(f32)], axis=1)
        common[f"WW{li}"] = np.ascontiguousarray(WW).astype(_BF16)
        common[f"br{li}"] = np.tile(b[None, :], (P, 1)).astype(f32)
        common[f"albc{li}"] = np.tile(al.reshape(1, hd),
                                      (P, meta["MZT"])).astype(_BF16)

    feats = np.asarray(inputs["features"], f32)
    in_maps = []
    for c in range(n_cores):
        pc = per_core[c]
        fo = np.zeros((ng * P, 128), _BF16)
        fo[pc["srow"]] = feats[c * npc:(c + 1) * npc].astype(_BF16)
        in_maps.append({
            **common,
            "features_own": fo,
            "idx_lo": pc["idx_lo"], "idx_hi": pc["idx_hi"],
            "ohb": pc["ohb"],
        })
    return in_maps


def kernel(**inputs):
    from concourse import bass_utils

    src = np.asarray(inputs["src"]).astype(np.int64)
    dst = np.asarray(inputs["dst"]).astype(np.int64)
    n_nodes = np.asarray(inputs["features"]).shape[0]
    meta, per_core = _preprocess(src, dst, n_nodes, N_CORES)
    nc = _get_compiled(meta)
    in_maps = _make_in_maps(inputs, meta, per_core)
    n_cores = meta["n_cores"]
    res = bass_utils.run_bass_kernel_spmd(
        nc, in_maps, core_ids=list(range(n_cores)),
        trace=bool(last_run_info.get("trace", False)))
    last_run_info["exec_time_ns"] = res.exec_time_ns
    last_run_info["profile_json"] = res.profile_json
    last_run_info["res"] = res

    npc = meta["npc"]
    out = np.empty((n_nodes, OUT_DIM), np.float32)
    for c in range(n_cores):
        rows = per_core[c]["srow"]
        out[c * npc:(c + 1) * npc] = res.results[c]["out"][rows]
    return out

